# revision 30
# baseline (speedup 1.0000x reference)
"""Trainium2 Bass kernel for nn_L1OutUB_14422500180350 (L1OutUB loss).

Math
----
reference computes, with B=512, Y=128:
    mu     = relu(x @ w1_mu + b1_mu) @ w2_mu + b2_mu                  [B, Y]
    logvar = tanh(relu(x @ w1_lv + b1_lv) @ w2_lv + b2_lv)            [B, Y]
    iv     = exp(-logvar)
    positive_i   = sum_k(-0.5*(mu_ik - y_ik)^2 iv_ik - 0.5*lv_ik)     [B]
    all_probs_ij = sum_k(-0.5*(y_jk - mu_ik)^2 iv_ik - 0.5*lv_ik)     [B, B]
    logits[a,i,j] = all_probs[i,j] + diag_mask[a,i]   (diag_mask [B,B,1])
    negative = logsumexp(logits, axis=0) - log(B-1)
    loss = (positive[None,:] - negative).mean()

The logsumexp summand depends on `a` only through diag_mask[a,i], so it
collapses exactly:  negative[i,j] = all_probs[i,j] + C  with
C = log(B-1+e^-20) - log(B-1)  (~4.03e-12).  Further,
mean_j (y_jk - mu_ik)^2 = (mu_ik - my_k)^2 + vary_k  with
my = mean_j y, vary = mean_j y^2 - my^2 (exact).  Hence

    loss = -0.5*(mean_i s_pos_i - mean_i s_neg_i) - C
    s_pos_i = sum_k[(mu_ik - y_ik)^2 iv_ik + lv_ik]
    s_neg_i = sum_k[((mu_ik - my_k)^2 + vary_k) iv_ik + lv_ik]

Distribution: data-parallel over batch rows, 64 rows per core on 8
NeuronCores; MLP weights replicated (spec sharding_hint).  Each core
returns its 64 s_pos and 64 s_neg row sums; the host combines in f64.

Layout: everything transposed on-chip (partition dim = feature dim) so
biases and y-moments are per-partition scalars, the two MLP layers chain
on the PE without any transpose, and the final k-sums are ones-vector
matmuls.

Raw Bass (not Tile): walrus cannot attach more than one fused sem wait
to an f32 Matmult (one-slot S3_LW struct), which Tile's auto-sync trips
over; standalone wait_ge instructions have no such limit, and we skip
Tile's multi-microsecond drain/barrier tail.

PSUM bank plan (one matmul group per bank, no reuse, so no PE-write /
ACT-read same-bank hazards beyond the sem-ordered L1 ping-pong):
  bank0/1: L1 mu-head chunks (ping-pong m%2)   bank2/3: L1 lv-head
  bank4: L2 mu   bank5: L2 lv   bank6: reduce s_pos   bank7: reduce s_neg

Semaphores: dS (DMA, +16/transfer), pS (PE group done), aS (ACT op
done), vS (DVE milestones).
"""

from contextlib import ExitStack

import numpy as np

import concourse.bass as bass
from concourse import mybir
from concourse.bass_utils import run_bass_kernel_spmd

B, X_DIM, Y_DIM, H2 = 512, 768, 128, 512
N_CORES = 8
RB = B // N_CORES  # 64 batch rows per core
KT = X_DIM // 128  # 6 k-tiles over the input dim
MT = H2 // 128  # 4 chunks over the hidden dim
F32 = mybir.dt.float32
AF = mybir.ActivationFunctionType
ALU = mybir.AluOpType
AX = mybir.AxisListType

# DVE tick milestones (asserted inside the vector block)
VS_SPOS = 12
VS_SNEG = 17
VS_DONE = 19


def build_nc() -> bass.Bass:
    nc = bass.Bass("TRN2", target_bir_lowering=False, debug=False)

    xsT = nc.dram_tensor("xsT", [X_DIM, RB], F32, kind="ExternalInput").ap()
    yT = nc.dram_tensor("yT", [Y_DIM, B], F32, kind="ExternalInput").ap()
    ysT = nc.dram_tensor("ysT", [Y_DIM, RB], F32, kind="ExternalInput").ap()
    w1m = nc.dram_tensor("w1m", [X_DIM, H2], F32, kind="ExternalInput").ap()
    w1l = nc.dram_tensor("w1l", [X_DIM, H2], F32, kind="ExternalInput").ap()
    w2m = nc.dram_tensor("w2m", [H2, Y_DIM], F32, kind="ExternalInput").ap()
    w2l = nc.dram_tensor("w2l", [H2, Y_DIM], F32, kind="ExternalInput").ap()
    consts = nc.dram_tensor("consts", [128, 10], F32, kind="ExternalInput").ap()
    out = nc.dram_tensor("out", [1, 2 * RB], F32, kind="ExternalOutput").ap()

    with ExitStack() as ctx:
        e = ctx.enter_context
        # ---- SBUF ----
        w1_sb = {
            "m": e(nc.sbuf_tensor("w1m_sb", [128, KT, H2], F32)),
            "l": e(nc.sbuf_tensor("w1l_sb", [128, KT, H2], F32)),
        }
        w2_sb = {
            "m": e(nc.sbuf_tensor("w2m_sb", [128, MT, Y_DIM], F32)),
            "l": e(nc.sbuf_tensor("w2l_sb", [128, MT, Y_DIM], F32)),
        }
        xsT_sb = e(nc.sbuf_tensor([128, KT, RB], F32))
        yT_sb = e(nc.sbuf_tensor([Y_DIM, B], F32))
        ysT_sb = e(nc.sbuf_tensor([Y_DIM, RB], F32))
        consts_sb = e(nc.sbuf_tensor([128, 10], F32))
        ones = e(nc.sbuf_tensor([128, 1], F32))
        h1_sb = {
            "m": e(nc.sbuf_tensor("h1m_sb", [128, MT, RB], F32)),
            "l": e(nc.sbuf_tensor("h1l_sb", [128, MT, RB], F32)),
        }
        muT = e(nc.sbuf_tensor([Y_DIM, RB], F32))
        lvT = e(nc.sbuf_tensor([Y_DIM, RB], F32))
        ivT = e(nc.sbuf_tensor([Y_DIM, RB], F32))
        y2 = e(nc.sbuf_tensor([Y_DIM, B], F32))
        my = e(nc.sbuf_tensor([Y_DIM, 1], F32))
        my2 = e(nc.sbuf_tensor([Y_DIM, 1], F32))
        vary = e(nc.sbuf_tensor([Y_DIM, 1], F32))
        mysq = e(nc.sbuf_tensor([Y_DIM, 1], F32))
        d = e(nc.sbuf_tensor([Y_DIM, RB], F32))
        d2 = e(nc.sbuf_tensor([Y_DIM, RB], F32))
        p1 = e(nc.sbuf_tensor([Y_DIM, RB], F32))
        v = e(nc.sbuf_tensor([Y_DIM, RB], F32))
        v2 = e(nc.sbuf_tensor([Y_DIM, RB], F32))
        u = e(nc.sbuf_tensor([Y_DIM, RB], F32))
        p2 = e(nc.sbuf_tensor([Y_DIM, RB], F32))
        s_pos = e(nc.sbuf_tensor([Y_DIM, RB], F32))
        s_neg = e(nc.sbuf_tensor([Y_DIM, RB], F32))
        out_sb = e(nc.sbuf_tensor([1, 2 * RB], F32))
        # ---- PSUM: one [128, 512] f32 tensor == exactly one 2KB bank ----
        banks = [
            e(nc.psum_tensor(f"bank{i}", [128, 512], F32)) for i in range(8)
        ]

        # HWDGE queues complete out of order → one semaphore per DMA
        dY = e(nc.semaphore("dY"))
        dC = e(nc.semaphore("dC"))
        dX = e(nc.semaphore("dX"))
        dW1m = e(nc.semaphore("dW1m"))
        dW1l = e(nc.semaphore("dW1l"))
        dW2m = e(nc.semaphore("dW2m"))
        dW2l = e(nc.semaphore("dW2l"))
        dYs = e(nc.semaphore("dYs"))
        dOut = e(nc.semaphore("dOut"))
        pS = e(nc.semaphore("pS"))
        aS = e(nc.semaphore("aS"))
        vS = e(nc.semaphore("vS"))
        dW1 = {"m": dW1m, "l": dW1l}
        dW2 = {"m": dW2m, "l": dW2l}

        with nc.Block() as block:

            @block.sync
            def _(sync):
                sync.dma_start(out=yT_sb[:, :], in_=yT).then_inc(dY, 16)
                sync.dma_start(out=consts_sb[:, :], in_=consts).then_inc(dC, 16)
                sync.dma_start(
                    out=xsT_sb[:, :, :], in_=xsT.rearrange("(t p) i -> p t i", p=128)
                ).then_inc(dX, 16)
                sync.dma_start(
                    out=w1_sb["m"][:, :, :],
                    in_=w1m.rearrange("(t p) h -> p t h", p=128),
                ).then_inc(dW1m, 16)
                sync.dma_start(
                    out=w1_sb["l"][:, :, :],
                    in_=w1l.rearrange("(t p) h -> p t h", p=128),
                ).then_inc(dW1l, 16)
                sync.dma_start(
                    out=w2_sb["m"][:, :, :],
                    in_=w2m.rearrange("(t p) n -> p t n", p=128),
                ).then_inc(dW2m, 16)
                sync.dma_start(
                    out=w2_sb["l"][:, :, :],
                    in_=w2l.rearrange("(t p) n -> p t n", p=128),
                ).then_inc(dW2l, 16)
                sync.dma_start(out=ysT_sb[:, :], in_=ysT).then_inc(dYs, 16)
                # final result out
                sync.wait_ge(vS, VS_DONE)
                sync.dma_start(out=out, in_=out_sb[:, :]).then_inc(dOut, 16)
                sync.wait_ge(dOut, 16)

            @block.tensor
            def _(tensor):
                # ---- layer 1, both heads; psum ping-pong between 2 banks/head
                tensor.wait_ge(dX, 16)
                for hi, head in enumerate(("m", "l")):
                    tensor.wait_ge(dW1[head], 16)
                    for m in range(MT):
                        if m >= 2:
                            tensor.wait_ge(aS, 4 * hi + m - 1)  # bank free (relu done)
                        ps = banks[2 * hi + (m % 2)][:, 0:RB]
                        for t in range(KT):
                            mm = nc.tensor.matmul(
                                ps,
                                w1_sb[head][:, t, m * 128 : (m + 1) * 128],
                                xsT_sb[:, t, :],
                                start=(t == 0),
                                stop=(t == KT - 1),
                            )
                        mm.then_inc(pS, 1)  # pS: 1..4 (m), 5..8 (l)
                # ---- layer 2 ----
                for hi, head in enumerate(("m", "l")):
                    tensor.wait_ge(dW2[head], 16)
                    tensor.wait_ge(aS, 4 + 4 * hi)  # all 4 h1T chunks relu'd
                    ps = banks[4 + hi][:, 0:RB]
                    for m in range(MT):
                        mm = nc.tensor.matmul(
                            ps,
                            w2_sb[head][:, m, :],
                            h1_sb[head][:, m, :],
                            start=(m == 0),
                            stop=(m == MT - 1),
                        )
                    mm.then_inc(pS, 1)  # 9 (mu), 10 (lv)
                # ---- partition reductions ----
                tensor.wait_ge(vS, VS_SPOS)  # s_pos ready (also covers ones)
                nc.tensor.matmul(
                    banks[6][0:1, 0:RB], ones[:, :], s_pos[:, :], start=True, stop=True
                ).then_inc(pS, 1)  # 11
                tensor.wait_ge(vS, VS_SNEG)  # s_neg ready
                nc.tensor.matmul(
                    banks[7][0:1, 0:RB], ones[:, :], s_neg[:, :], start=True, stop=True
                ).then_inc(pS, 1)  # 12

            @block.scalar
            def _(scalar):
                scalar.wait_ge(dC, 16)  # consts (biases)
                # L1 relu(+bias): consume psum banks as PE finishes them
                for hi, head in enumerate(("m", "l")):
                    for m in range(MT):
                        scalar.wait_ge(pS, 4 * hi + m + 1)
                        nc.scalar.activation(
                            out=h1_sb[head][:, m, :],
                            in_=banks[2 * hi + (m % 2)][:, 0:RB],
                            func=AF.Relu,
                            bias=consts_sb[:, 5 * hi + m : 5 * hi + m + 1],
                            scale=1.0,
                        ).then_inc(aS, 1)  # aS: 1..4 (m), 5..8 (l)
                scalar.wait_ge(pS, 9)
                nc.scalar.activation(
                    out=muT[:, :],
                    in_=banks[4][:, 0:RB],
                    func=AF.Identity,
                    bias=consts_sb[:, 4:5],
                    scale=1.0,
                ).then_inc(aS, 1)  # 9
                scalar.wait_ge(pS, 10)
                nc.scalar.activation(
                    out=lvT[:, :],
                    in_=banks[5][:, 0:RB],
                    func=AF.Tanh,
                    bias=consts_sb[:, 9:10],
                    scale=1.0,
                ).then_inc(aS, 1)  # 10
                scalar.wait_ge(aS, 10)  # same-engine RAW: lvT visible
                nc.scalar.activation(
                    out=ivT[:, :], in_=lvT[:, :], func=AF.Exp, scale=-1.0
                ).then_inc(aS, 1)  # 11

            @block.vector
            def _(vector):
                # Every op bumps vS; same-engine RAW consumers wait the
                # producer's tick (deep-pipeline visibility rule).  A
                # python counter keeps tick bookkeeping consistent.
                tick = {"v": 0}

                def bump(inst):
                    inst.then_inc(vS, 1)
                    tick["v"] += 1
                    return tick["v"]

                vector.wait_ge(dY, 16)  # yT
                bump(nc.vector.memset(ones[:, :], 1.0))
                t_my = bump(nc.vector.reduce_sum(my[:, :], yT_sb[:, :], axis=AX.X))
                t_y2 = bump(nc.vector.tensor_mul(y2[:, :], yT_sb[:, :], yT_sb[:, :]))
                vector.wait_ge(vS, t_y2)  # covers t_my too
                t_my2 = bump(nc.vector.reduce_sum(my2[:, :], y2[:, :], axis=AX.X))
                t_mym = bump(
                    nc.vector.tensor_scalar_mul(my[:, :], my[:, :], 1.0 / B)
                )
                vector.wait_ge(vS, t_my2)
                t_my2m = bump(
                    nc.vector.tensor_scalar_mul(my2[:, :], my2[:, :], 1.0 / B)
                )
                vector.wait_ge(vS, t_mym)
                t_mysq = bump(nc.vector.tensor_mul(mysq[:, :], my[:, :], my[:, :]))
                vector.wait_ge(vS, t_mysq)  # covers t_my2m
                t_vary = bump(
                    nc.vector.tensor_sub(vary[:, :], my2[:, :], mysq[:, :])
                )
                # s_pos = (mu - ys)^2 * iv + lv
                vector.wait_ge(dYs, 16)  # ysT
                vector.wait_ge(aS, 11)  # muT, lvT, ivT all ready
                t_d = bump(nc.vector.tensor_sub(d[:, :], muT[:, :], ysT_sb[:, :]))
                vector.wait_ge(vS, t_d)
                t_d2 = bump(nc.vector.tensor_mul(d2[:, :], d[:, :], d[:, :]))
                vector.wait_ge(vS, t_d2)
                t_p1 = bump(nc.vector.tensor_mul(p1[:, :], d2[:, :], ivT[:, :]))
                vector.wait_ge(vS, t_p1)
                t_spos = bump(
                    nc.vector.tensor_add(s_pos[:, :], p1[:, :], lvT[:, :])
                )
                # s_neg = ((mu - my)^2 + vary) * iv + lv
                t_v = bump(
                    nc.vector.tensor_scalar_sub(v[:, :], muT[:, :], my[:, :])
                )
                vector.wait_ge(vS, t_v)
                t_v2 = bump(nc.vector.tensor_mul(v2[:, :], v[:, :], v[:, :]))
                vector.wait_ge(vS, t_v2)
                t_u = bump(
                    nc.vector.tensor_scalar_add(u[:, :], v2[:, :], vary[:, :])
                )
                vector.wait_ge(vS, t_u)
                t_p2 = bump(nc.vector.tensor_mul(p2[:, :], u[:, :], ivT[:, :]))
                vector.wait_ge(vS, t_p2)
                t_sneg = bump(
                    nc.vector.tensor_add(s_neg[:, :], p2[:, :], lvT[:, :])
                )
                # collect reduce results
                vector.wait_ge(pS, 11)
                bump(nc.vector.tensor_copy(out_sb[:, 0:RB], banks[6][0:1, 0:RB]))
                vector.wait_ge(pS, 12)
                bump(
                    nc.vector.tensor_copy(
                        out_sb[:, RB : 2 * RB], banks[7][0:1, 0:RB]
                    )
                )
                # final tick must be VS_DONE; PE red waits use VS_SPOS/VS_SNEG
                assert tick["v"] == VS_DONE, tick
                assert (t_spos, t_sneg) == (VS_SPOS, VS_SNEG), (t_spos, t_sneg)

    return nc


def make_in_maps(inputs: dict) -> list[dict]:
    f = lambda a: np.ascontiguousarray(np.asarray(a, dtype=np.float32))
    x = f(inputs["x_samples"])
    y = f(inputs["y_samples"])
    xT = f(x.T)  # [768, 512]
    yT = f(y.T)  # [128, 512]
    consts = np.zeros((128, 10), np.float32)
    consts[:, 0:4] = f(inputs["b1_mu"]).reshape(4, 128).T
    consts[:, 4] = f(inputs["b2_mu"])
    consts[:, 5:9] = f(inputs["b1_lv"]).reshape(4, 128).T
    consts[:, 9] = f(inputs["b2_lv"])
    w1m, w1l = f(inputs["w1_mu"]), f(inputs["w1_lv"])
    w2m, w2l = f(inputs["w2_mu"]), f(inputs["w2_lv"])
    in_maps = []
    for c in range(N_CORES):
        sl = slice(c * RB, (c + 1) * RB)
        in_maps.append(
            {
                "xsT": f(xT[:, sl]),
                "yT": yT,
                "ysT": f(yT[:, sl]),
                "w1m": w1m,
                "w1l": w1l,
                "w2m": w2m,
                "w2l": w2l,
                "consts": consts,
            }
        )
    return in_maps


def combine(results: list[dict]) -> np.float32:
    pos = np.concatenate(
        [results[c]["out"][0, :RB] for c in range(N_CORES)]
    ).astype(np.float64)
    neg = np.concatenate(
        [results[c]["out"][0, RB:] for c in range(N_CORES)]
    ).astype(np.float64)
    C = np.log(B - 1.0 + np.exp(-20.0)) - np.log(B - 1.0)
    loss = -0.5 * (pos.mean() - neg.mean()) - C
    return np.float32(loss)


_NC_CACHE = None


def run(inputs: dict, **spmd_kwargs):
    """Build (cached), run on 8 cores, return (loss, BassKernelResults)."""
    global _NC_CACHE
    if _NC_CACHE is None:
        _NC_CACHE = build_nc()
    bkr = run_bass_kernel_spmd(
        _NC_CACHE, make_in_maps(inputs), list(range(N_CORES)), **spmd_kwargs
    )
    return combine(bkr.results), bkr


def kernel(**inputs) -> np.float32:
    loss, _ = run(inputs)
    return loss
